# revision 1
# baseline (speedup 1.0000x reference)
"""APPNP conv kernel for 8 TRN2 NeuronCores — HBM-gather redesign.

out = 0.8 * spmm(adj, h) + 0.2 * h0
  spmm: out[i] = sum_{e: row[e]==i} vals[e] * h[col[e]],  N=100000, E=1.6M, d=64

Per core (nodes row-partitioned 12500/core, no collectives):
  host: permute rows into degree-balanced 32-row windows; windows grouped
  (NGRP groups); per group, distinct cols are packed PACK-per-quad into an
  HBM table [n_pack, PACK*64] bf16 (first-use order, shared across windows
  of the group); per (window, pack touched) one gather slot.
  device: gpsimd dma_gather (HBM source, non-transpose) pulls each slot's
  512B quad directly into matmul-ready layout [slot%128 partition, free];
  PE reduces each 128-slot tile into its windows' PSUM rows via PACK
  val-carrying selection matmuls per (tile, window); scalar evicts
  0.8*psum into stage; vector adds 0.2*h0; one output DMA.
"""
import sys
sys.path.insert(0, "/opt/trn_rl_repo")

import numpy as np
import ml_dtypes

import concourse.bacc as bacc
import concourse.bass as bass
import concourse.mybir as mybir
from concourse import bass_utils
from concourse.library_config import mlp
from concourse._compat import cdiv

N_NODES = 100000
ALPHA = 0.2
D = 64
CORES = 8
WIN = 32                  # rows per window (matmul M)
PACK = 4                  # h rows per gather element (PACK*128B)
NGRP = 16                 # groups (dedup scope / gather tables) per core
BATCH_TILES = 64          # max tiles per dma_gather instruction
NSLOT = 32                # psum block slots in flight
NPC = N_NODES // CORES            # 12500
NPC_PAD = cdiv(NPC, 128) * 128    # 12544
NBLK = NPC_PAD // 128             # 98
NWIN = NPC_PAD // WIN             # 392
WPG = cdiv(NWIN, NGRP)            # windows per group


# ----------------------------------------------------------------- host prep
def _prep_core(edge_row, edge_col, edge_vals, k):
    lo = np.searchsorted(edge_row, k * NPC)
    hi = np.searchsorted(edge_row, (k + 1) * NPC)
    rows = edge_row[lo:hi] - k * NPC
    cols = edge_col[lo:hi].astype(np.int64)
    vals = edge_vals[lo:hi].astype(np.float32)

    deg = np.bincount(rows, minlength=NPC_PAD)
    order = np.argsort(-deg, kind="stable")
    half = NPC_PAD // 2
    perm = np.empty(NPC_PAD, dtype=np.int64)
    perm[0::2] = order[:half]
    perm[1::2] = order[half:][::-1]
    slot_of = np.empty(NPC_PAD, dtype=np.int64)
    slot_of[perm] = np.arange(NPC_PAD)

    es = slot_of[rows]
    eo = np.argsort(es, kind="stable")
    e_slot = es[eo]
    e_col = cols[eo]
    e_val = vals[eo]
    e_win = e_slot // WIN

    groups = []
    for g in range(NGRP):
        wlo, whi = g * WPG, min((g + 1) * WPG, NWIN)
        elo = np.searchsorted(e_win, wlo)
        ehi = np.searchsorted(e_win, whi)
        w = e_win[elo:ehi]
        c = e_col[elo:ehi]
        v = e_val[elo:ehi]
        r = e_slot[elo:ehi] % WIN
        if len(w) == 0:
            groups.append(dict(n_pack=0, pack_cols=np.zeros((0,), np.int64),
                               slot_w=np.zeros((0,), np.int64),
                               slot_pack=np.zeros((0,), np.int64),
                               e_slotidx=np.zeros((0,), np.int64),
                               e_lane=np.zeros((0,), np.int64),
                               e_row=np.zeros((0,), np.int64),
                               e_val=np.zeros((0,), np.float32)))
            continue
        key = w * (1 << 20) + c
        uk, inv_inc = np.unique(key, return_inverse=True)
        inc_w = uk >> 20
        inc_c = uk & ((1 << 20) - 1)
        # fresh incidences: first (w,c)-ordered occurrence of each col
        ff = np.sort(np.unique(inc_c, return_index=True)[1])
        fcols = inc_c[ff]
        pack_of_col = np.full(N_NODES, -1, dtype=np.int64)
        lane_of_col = np.zeros(N_NODES, dtype=np.int64)
        pk = np.arange(len(ff))
        pack_of_col[fcols] = pk // PACK
        lane_of_col[fcols] = pk % PACK
        n_pack = cdiv(len(ff), PACK)
        inc_pack = pack_of_col[inc_c]
        skey = inc_w * (1 << 20) + inc_pack
        suk, inv_slot = np.unique(skey, return_inverse=True)
        slot_w = suk >> 20
        slot_pack = suk & ((1 << 20) - 1)
        e_slotidx = inv_slot[inv_inc]      # per edge -> group slot index
        e_lane = lane_of_col[c]
        groups.append(dict(n_pack=n_pack, pack_cols=fcols,
                           slot_w=slot_w, slot_pack=slot_pack,
                           e_slotidx=e_slotidx, e_lane=e_lane,
                           e_row=r, e_val=v))
    return dict(perm=perm, groups=groups)


def _preprocess(edge_row, edge_col, edge_vals, h, h0):
    edge_row = np.asarray(edge_row)
    edge_col = np.asarray(edge_col)
    edge_vals = np.asarray(edge_vals, dtype=np.float32)
    h_bf = np.asarray(h, dtype=np.float32).astype(ml_dtypes.bfloat16)
    h0 = np.asarray(h0, dtype=np.float32)

    cores = [_prep_core(edge_row, edge_col, edge_vals, k) for k in range(CORES)]

    # common slot schedule: per window, max slot count across cores;
    # >=128 so a tile overlaps at most 2 windows (full-K parity-stripe
    # matmuls need no finer alignment)
    slots_w = np.ones(NWIN, dtype=np.int64)
    for c in cores:
        for g, gd in enumerate(c["groups"]):
            if len(gd["slot_w"]):
                cnt = np.bincount(gd["slot_w"], minlength=NWIN)
                slots_w = np.maximum(slots_w, cnt)
    slots_w = np.maximum(slots_w, 128)

    # group tile layout: each group padded to whole tiles
    grp_tiles = []
    win_pos = np.zeros(NWIN, dtype=np.int64)   # global slot pos of window start
    tile0_of_grp = []
    t0 = 0
    for g in range(NGRP):
        wlo, whi = g * WPG, min((g + 1) * WPG, NWIN)
        if wlo >= NWIN:
            tile0_of_grp.append(t0)
            grp_tiles.append(0)
            continue
        cnt = slots_w[wlo:whi]
        pos = np.cumsum(cnt) - cnt + t0 * 128
        win_pos[wlo:whi] = pos
        ntl = cdiv(int(cnt.sum()), 128)
        tile0_of_grp.append(t0)
        grp_tiles.append(ntl)
        t0 += ntl
    T = t0

    # batches: per group, tiles split into <=BATCH_TILES chunks
    batches = []       # (group, tile_start, ntl)
    for g in range(NGRP):
        left = grp_tiles[g]
        ts = tile0_of_grp[g]
        while left > 0:
            n = min(BATCH_TILES, left)
            batches.append((g, ts, n))
            ts += n
            left -= n

    # matmul schedule: every window has >=128 slots so a tile overlaps at
    # most 2 windows; window w uses rv parity stripe w%2, full-K matmuls.
    tile_wins = [[] for _ in range(T)]
    for w in range(NWIN):
        p0 = int(win_pos[w])
        p1 = p0 + int(slots_w[w])
        for t in range(p0 // 128, (p1 - 1) // 128 + 1):
            tile_wins[t].append(w)
    for t in range(T):
        assert len(tile_wins[t]) <= 2, (t, tile_wins[t])
    n_mm = 0
    win_last_mm = {}
    win_mm_total = {}
    mm_of_tile_end = np.zeros(T, dtype=np.int64)
    for t in range(T):
        for w in tile_wins[t]:
            n_mm += PACK
            win_last_mm[w] = n_mm
            win_mm_total[w] = win_mm_total.get(w, 0) + PACK
        mm_of_tile_end[t] = n_mm
    blk_last_mm = {}
    for w in range(NWIN):
        b = w // (128 // WIN)
        blk_last_mm[b] = max(blk_last_mm.get(b, 0), win_last_mm[w])
    n_slots_total = int(slots_w.sum())

    n_pack_max = [max(max(c["groups"][g]["n_pack"] for c in cores), 1)
                  for g in range(NGRP)]

    # ---- per-core tensors
    in_maps = []
    for k in range(CORES):
        c = cores[k]
        im = {}
        rv = np.zeros((128, T * 2 * PACK * WIN), dtype=np.float32)
        gidx_flat = np.zeros(T * 128, dtype=np.int64)
        for g in range(NGRP):
            gd = c["groups"][g]
            npk = n_pack_max[g]
            tbl = np.zeros((npk, PACK * 64), dtype=ml_dtypes.bfloat16)
            ncol = len(gd["pack_cols"])
            if ncol:
                rowsel = h_bf[gd["pack_cols"]]     # [ncol, 64]
                pk = np.arange(ncol)
                tbl.reshape(npk, PACK, 64)[pk // PACK, pk % PACK] = rowsel
            im[f"tbl{g}"] = tbl
            if len(gd["slot_w"]) == 0:
                continue
            # slot global positions: rank within window + win_pos
            sw = gd["slot_w"]
            # rank within window: slots sorted by (w, pack) -> cumcount
            starts = np.searchsorted(sw, np.arange(NWIN))
            rank = np.arange(len(sw)) - starts[sw]
            pos = win_pos[sw] + rank
            gidx_flat[pos] = gd["slot_pack"]
            ep = pos[gd["e_slotidx"]]
            stripe = sw[gd["e_slotidx"]] % 2
            np.add.at(rv, (ep % 128,
                           (ep // 128) * (2 * PACK * WIN) + stripe * (PACK * WIN)
                           + gd["e_lane"] * WIN + gd["e_row"]),
                      gd["e_val"])

        # wrap gidx per batch
        gidx = np.zeros((128, T * 8), dtype=np.int16)
        for (g, ts, ntl) in batches:
            n_idx = ntl * 128
            blk = gidx_flat[ts * 128: ts * 128 + n_idx]
            wrapped = blk.reshape(n_idx // 16, 16).T.astype(np.int16)
            for rr in range(8):
                gidx[16 * rr:16 * (rr + 1), ts * 8: ts * 8 + n_idx // 16] = wrapped
        im["gidx"] = gidx
        im["rv"] = rv.astype(ml_dtypes.bfloat16)

        # h0 in permuted block layout
        perm = c["perm"]
        gl = perm + k * NPC
        valid = perm < NPC
        h0p = np.zeros((128, NBLK * 64), dtype=np.float32)
        slot_idx = np.arange(NPC_PAD)
        vs = slot_idx[valid]
        h0p[(vs % 128)[:, None],
            ((vs // 128) * 64)[:, None] + np.arange(64)[None, :]] = h0[gl[valid]]
        im["h0p"] = h0p
        in_maps.append(im)

    meta = dict(T=T, batches=batches, tile_wins=tile_wins,
                win_last_mm=win_last_mm, blk_last_mm=blk_last_mm,
                win_mm_total=win_mm_total,
                mm_of_tile_end=mm_of_tile_end, n_mm=n_mm,
                n_pack_max=n_pack_max, n_slots=n_slots_total,
                perms=[c["perm"] for c in cores])
    return in_maps, meta


# ------------------------------------------------------------- numpy device sim
def _simulate(in_maps, meta, h0_unused=None):
    """Simulate the device schedule exactly (matmul semantics) in numpy."""
    T = meta["T"]
    outs = []
    for k in range(CORES):
        im = in_maps[k]
        rv = np.asarray(im["rv"], dtype=np.float32)
        # unwrap gidx
        gidx_flat = np.zeros(T * 128, dtype=np.int64)
        for (g, ts, ntl) in meta["batches"]:
            n_idx = ntl * 128
            wrapped = im["gidx"][0:16, ts * 8: ts * 8 + n_idx // 16]
            gidx_flat[ts * 128: ts * 128 + n_idx] = wrapped.T.reshape(-1)
        # gather + matmuls
        psum = np.zeros((NWIN, WIN, 64), dtype=np.float32)
        for (g, ts, ntl) in meta["batches"]:
            tbl = np.asarray(im[f"tbl{g}"], dtype=np.float32)
            for t in range(ts, ts + ntl):
                elems = tbl[gidx_flat[t * 128:(t + 1) * 128]]  # [128, PACK*64]
                for w in meta["tile_wins"][t]:
                    for q in range(PACK):
                        rvc = PACK * WIN
                        base = t * 2 * rvc + (w % 2) * rvc
                        lhsT = rv[:, base + q * WIN: base + q * WIN + WIN]
                        rhs = elems[:, q * 64:(q + 1) * 64]
                        psum[w] += lhsT.T @ rhs
        stage = np.zeros((128, NBLK * 64), dtype=np.float32)
        for b in range(NBLK):
            blk = np.concatenate([psum[b * (128 // WIN) + j] for j in range(128 // WIN)], axis=0)
            stage[:, b * 64:(b + 1) * 64] = (1.0 - ALPHA) * blk
        stage += ALPHA * np.asarray(im["h0p"], dtype=np.float32)
        outs.append(stage)
    return outs


def assemble(outs, meta):
    out = np.zeros((N_NODES, D), dtype=np.float32)
    for k in range(CORES):
        o = np.asarray(outs[k], dtype=np.float32)
        o = o.reshape(128, NBLK, D).transpose(1, 0, 2).reshape(-1, D)
        perm = meta["perms"][k]
        valid = perm < NPC
        out[perm[valid] + k * NPC] = o[valid]
    return out


# ------------------------------------------------------------- graph builder
from contextlib import ExitStack

RVC = 2 * PACK * WIN      # rv columns per tile (2 parity stripes)
STR = PACK * WIN          # stripe width
QPW = 128 // WIN          # windows per psum block


def _build(meta, reps=1, cut=None, stage="full", ev_cut=None):
    T = meta["T"]
    batches = meta["batches"]
    mm_of_tile_end = meta["mm_of_tile_end"]
    if cut is not None:
        batches = batches[:cut]
    tile_wins = meta["tile_wins"]
    n_mm = meta["n_mm"]
    blk_last_mm = meta["blk_last_mm"]
    n_pack_max = meta["n_pack_max"]
    bf16 = mybir.dt.bfloat16
    f32 = mybir.dt.float32
    NB = len(batches)
    NRD = 8

    # per-batch cumulative matmul count (within one rep)
    cum_mm = []
    acc = 0
    for (g, ts, ntl) in batches:
        for t in range(ts, ts + ntl):
            acc += PACK * len(tile_wins[t])
        cum_mm.append(acc)
    n_mm = acc

    blk_last_mm = {b: v for b, v in blk_last_mm.items() if v <= n_mm}
    evict_blocks = sorted(blk_last_mm)
    if ev_cut is not None:
        evict_blocks = evict_blocks[:ev_cut]
        blk_last_mm = {b: blk_last_mm[b] for b in evict_blocks}
    n_ev = len(evict_blocks)
    ev_ord = {b: i for i, b in enumerate(evict_blocks)}

    def rd_target(rep, local):
        """(sem index, wait value) for matmul count `local` of rep `rep`."""
        return rep % NRD, n_mm * (rep // NRD) + local

    cut_mm = cum_mm_total = None

    nc = bacc.Bacc("TRN2")
    tbl_hbm = [nc.declare_dram_parameter(f"tbl{g}", [n_pack_max[g], PACK * 64],
                                         bf16, isOutput=False)
               for g in range(NGRP)]
    gidx_hbm = nc.declare_dram_parameter("gidx", [128, T * 8], mybir.dt.int16,
                                         isOutput=False)
    rv_hbm = nc.declare_dram_parameter("rv", [128, T * RVC], bf16, isOutput=False)
    h0p_hbm = nc.declare_dram_parameter("h0p", [128, NBLK * 64], f32,
                                        isOutput=False)
    out_hbm = nc.declare_dram_parameter("out", [128, NBLK * 64], f32,
                                        isOutput=True)

    with ExitStack() as ctx:
        block = ctx.enter_context(nc.Block())
        gidxb = ctx.enter_context(nc.sbuf_tensor("gidxb", [128, T * 8], mybir.dt.int16))
        rvb = [ctx.enter_context(nc.sbuf_tensor(f"rvb{j}", [128, BATCH_TILES * RVC], bf16))
               for j in range(2)]
        gbuf = [ctx.enter_context(nc.sbuf_tensor(f"gbuf{j}", [128, BATCH_TILES, PACK * 64], bf16))
                for j in range(2)]
        h0s = ctx.enter_context(nc.sbuf_tensor("h0s", [128, NBLK * 64], f32))
        stage_sb = ctx.enter_context(nc.sbuf_tensor("stage", [128, NBLK * 64], f32))
        pso = [ctx.enter_context(nc.psum_tensor(f"pso{j}", [128, 512], f32))
               for j in range(4)]
        s_in = ctx.enter_context(nc.semaphore("s_in"))
        s_rv = [ctx.enter_context(nc.semaphore(f"s_rv{j}")) for j in range(2)]
        s_ga = [ctx.enter_context(nc.semaphore(f"s_ga{j}")) for j in range(2)]
        s_rd = [ctx.enter_context(nc.semaphore(f"s_rd{j}")) for j in range(NRD)]
        s_ae = ctx.enter_context(nc.semaphore("s_ae"))
        s_h0 = ctx.enter_context(nc.semaphore("s_h0"))

        # ---- sync: input DMAs, rv batch stream, final store
        CH = BATCH_TILES * RVC     # rv cols per chunk == 64 tiles
        NCH = cdiv(T, BATCH_TILES)  # chunks per rep
        chunk_last_mm = [int(mm_of_tile_end[min((c + 1) * BATCH_TILES, T) - 1])
                         for c in range(NCH)]

        @block.sync
        def _(s):
            s.dma_start(gidxb[:], gidx_hbm[:]).then_inc(s_in, 16)
            s.dma_start(h0s[:], h0p_hbm[:]).then_inc(s_in, 16)
            for r in range(reps):
                for c in range(NCH):
                    gc = r * NCH + c
                    lo = c * CH
                    hi = min(T * RVC, lo + CH)
                    if gc >= 2:
                        pr, pc = divmod(gc - 2, NCH)
                        sid, val = rd_target(pr, chunk_last_mm[pc])
                        s.wait_ge(s_rd[sid], val)
                    s.dma_start(rvb[gc % 2][:, 0:hi - lo],
                                rv_hbm[:, lo:hi]).then_inc(s_rv[gc % 2], 16)
            if stage == "full":
                s.wait_ge(s_h0, 1 + n_ev * reps)
            elif stage == "evict":
                s.wait_ge(s_ae, n_ev * reps)
            elif stage == "mm":
                sid, val = rd_target(reps - 1, n_mm)
                s.wait_ge(s_rd[sid], val)
            else:
                for gi in range(reps * NB - 2, reps * NB):
                    if gi >= 0:
                        s.wait_ge(s_ga[gi % 2], 16 * (gi // 2 + 1))
            s.dma_start(out_hbm[:], stage_sb[:]).then_inc(s_in, 16)
            s.wait_ge(s_in, 48)

        # ---- gpsimd: gathers
        @block.gpsimd
        def _(ge: bass.BassGpSimd):
            ge.load_library(mlp)
            ge.wait_ge(s_in, 32)
            for r in range(reps):
                for i, (g, ts, ntl) in enumerate(batches):
                    gi = r * NB + i
                    n_idx = ntl * 128
                    if gi >= 2:
                        pr, pl = divmod(gi - 2, NB)
                        sid, val = rd_target(pr, cum_mm[pl])
                        ge.wait_ge(s_rd[sid], val)
                    ge.dma_gather(
                        gbuf[gi % 2][:, 0:ntl, :],
                        tbl_hbm[g][:],
                        gidxb[:, ts * 8:ts * 8 + n_idx // 16],
                        n_idx, n_idx, PACK * 64,
                        transpose=False, single_packet=False,
                    ).then_inc(s_ga[gi % 2], 16)

        # ---- tensor: reduction matmuls
        if stage in ("mm", "evict", "full"):
            _tensor_gate = True
        else:
            _tensor_gate = False

        @block.tensor
        def _(te):
            if not _tensor_gate:
                return
            te.wait_ge(s_in, 32)
            for r in range(reps):
                mm = 0
                win_seen = {}
                bank_waited = set()
                chunk_seen = -1
                for i, (g, ts, ntl) in enumerate(batches):
                    gi = r * NB + i
                    te.wait_ge(s_ga[gi % 2], 16 * (gi // 2 + 1))
                    for j in range(ntl):
                        t = ts + j
                        tc = t // BATCH_TILES
                        if tc > chunk_seen:
                            chunk_seen = tc
                            gc = r * NCH + tc
                            te.wait_ge(s_rv[gc % 2], 16 * (gc // 2 + 1))
                        for w in tile_wins[t]:
                            b = w // QPW
                            q = w % QPW
                            bank = pso[b % 4]
                            c0 = ((b // 4) % 8) * 64
                            # PSUM bank guard: bank b%4 is free for block b only
                            # once block b-4 (same bank) has been evicted.
                            if stage in ("evict", "full") and b not in bank_waited:
                                bank_waited.add(b)
                                pb = b - 4
                                if pb in ev_ord:
                                    te.wait_ge(s_ae, r * n_ev + ev_ord[pb] + 1)
                            first = w not in win_seen
                            win_seen[w] = win_seen.get(w, 0) + PACK
                            last = win_seen[w] == meta["win_mm_total"][w]
                            base = (t % BATCH_TILES) * RVC + (w % 2) * STR
                            rvbuf = rvb[(r * NCH + t // BATCH_TILES) % 2]
                            for qq in range(PACK):
                                mm += 1
                                sid, val = rd_target(r, mm)
                                te.matmul(
                                    out=bank[WIN * q:WIN * q + WIN, c0:c0 + 64],
                                    lhsT=rvbuf[:, base + qq * WIN:
                                               base + qq * WIN + WIN],
                                    rhs=gbuf[gi % 2][:, j,
                                                     qq * 64:qq * 64 + 64],
                                    start=(first and qq == 0),
                                    stop=(last and qq == PACK - 1),
                                    tile_position=(0, WIN * q),
                                    skip_group_check=True,
                                ).then_inc(s_rd[sid], 1)

        # ---- scalar: block evictions (0.8 * psum -> stage)
        @block.scalar
        def _(sc):
            if stage not in ("evict", "full"):
                return
            for r in range(reps):
                if r > 0:
                    sc.wait_ge(s_h0, 1 + n_ev * r)
                for b in evict_blocks:
                    bank = pso[b % 4]
                    c0 = ((b // 4) % 8) * 64
                    sid, val = rd_target(r, blk_last_mm[b])
                    sc.wait_ge(s_rd[sid], val)
                    sc.activation(
                        out=stage_sb[:, b * 64:(b + 1) * 64],
                        in_=bank[:, c0:c0 + 64],
                        func=mybir.ActivationFunctionType.Copy,
                        scale=1.0 - ALPHA,
                    ).then_inc(s_ae, 1)

        # ---- vector: h0 prescale once, block h0-adds per rep
        @block.vector
        def _(v):
            if stage != "full":
                return
            v.wait_ge(s_in, 32)
            v.tensor_scalar(out=h0s[:], in0=h0s[:], scalar1=ALPHA, scalar2=None,
                            op0=mybir.AluOpType.mult).then_inc(s_h0, 1)
            v.wait_ge(s_h0, 1)
            for r in range(reps):
                for bi, b in enumerate(evict_blocks):
                    v.wait_ge(s_ae, r * n_ev + bi + 1)
                    v.tensor_tensor(
                        out=stage_sb[:, b * 64:(b + 1) * 64],
                        in0=stage_sb[:, b * 64:(b + 1) * 64],
                        in1=h0s[:, b * 64:(b + 1) * 64],
                        op=mybir.AluOpType.add,
                    ).then_inc(s_h0, 1)

    nc.compile()
    return nc


# ------------------------------------------------------- gather-chain bench
def _build_bench(meta, reps, with_rv=True, with_gather=True):
    """Gather + rv-stream pipeline only (the serial backbone), repeated."""
    T = meta["T"]
    batches = meta["batches"]
    n_pack_max = meta["n_pack_max"]
    bf16 = mybir.dt.bfloat16
    f32 = mybir.dt.float32
    NB = len(batches)
    NSG = 8

    nc = bacc.Bacc("TRN2")
    tbl_hbm = [nc.declare_dram_parameter(f"tbl{g}", [n_pack_max[g], PACK * 64],
                                         bf16, isOutput=False)
               for g in range(NGRP)]
    gidx_hbm = nc.declare_dram_parameter("gidx", [128, T * 8], mybir.dt.int16,
                                         isOutput=False)
    rv_hbm = nc.declare_dram_parameter("rv", [128, T * RVC], bf16, isOutput=False)
    out_hbm = nc.declare_dram_parameter("out", [128, 128], f32, isOutput=True)

    with ExitStack() as ctx:
        block = ctx.enter_context(nc.Block())
        gidxb = ctx.enter_context(nc.sbuf_tensor("gidxb", [128, T * 8], mybir.dt.int16))
        rvb = [ctx.enter_context(nc.sbuf_tensor(f"rvb{j}", [128, BATCH_TILES * RVC], bf16))
               for j in range(2)]
        gbuf = [ctx.enter_context(nc.sbuf_tensor(f"gbuf{j}", [128, BATCH_TILES, PACK * 64], bf16))
                for j in range(2)]
        stage = ctx.enter_context(nc.sbuf_tensor("stage", [128, 128], f32))
        s_in = ctx.enter_context(nc.semaphore("s_in"))
        s_rv = [ctx.enter_context(nc.semaphore(f"s_rv{j}")) for j in range(2)]
        s_ga = [ctx.enter_context(nc.semaphore(f"s_ga{j}")) for j in range(NSG)]

        @block.sync
        def _(s):
            s.dma_start(gidxb[:], gidx_hbm[:]).then_inc(s_in, 16)
            CHUNK = BATCH_TILES * RVC            # cols per chunk (32KB/part)
            total_cols = T * RVC
            NCH = (total_cols + CHUNK - 1) // CHUNK
            for r in range(reps):
                if not with_rv:
                    continue
                for ci in range(NCH):
                    gc = r * NCH + ci
                    lo = ci * CHUNK
                    hi = min(total_cols, lo + CHUNK)
                    if gc >= 2:
                        s.wait_ge(s_rv[(gc - 2) % 2], 16 * ((gc - 2) // 2 + 1))
                    s.dma_start(rvb[gc % 2][:, 0:hi - lo],
                                rv_hbm[:, lo:hi]).then_inc(s_rv[gc % 2], 16)
            if with_gather:
                for j in range(NSG):
                    tot = sum(1 for x in range(reps * NB) if x % NSG == j)
                    if tot:
                        s.wait_ge(s_ga[j], 16 * tot)
            if with_rv:
                for j in range(2):
                    tot = sum(1 for x in range(reps * NCH) if x % 2 == j)
                    if tot:
                        s.wait_ge(s_rv[j], 16 * tot)
            s.dma_start(out_hbm[:], stage[:]).then_inc(s_in, 16)
            s.wait_ge(s_in, 32)

        @block.gpsimd
        def _(ge: bass.BassGpSimd):
            if not with_gather:
                return
            ge.load_library(mlp)
            ge.wait_ge(s_in, 16)
            for r in range(reps):
                for i, (g, ts, ntl) in enumerate(batches):
                    gi = r * NB + i
                    n_idx = ntl * 128
                    if gi >= 2:
                        pgi = gi - 2
                        ge.wait_ge(s_ga[pgi % NSG], 16 * (pgi // NSG + 1))
                        pass
                    ge.dma_gather(
                        gbuf[gi % 2][:, 0:ntl, :],
                        tbl_hbm[g][:],
                        gidxb[:, ts * 8:ts * 8 + n_idx // 16],
                        n_idx, n_idx, PACK * 64,
                        transpose=False, single_packet=False,
                    ).then_inc(s_ga[gi % NSG], 16)

        @block.vector
        def _(v):
            v.wait_ge(s_in, 16)
            v.tensor_scalar(out=stage[:], in0=stage[:], scalar1=0.0, scalar2=None,
                            op0=mybir.AluOpType.mult)

    nc.compile()
    return nc


_CACHE = {}
LAST_META = None


def kernel(edge_row, edge_col, edge_vals, h, h0):
    global LAST_META
    in_maps, meta = _preprocess(edge_row, edge_col, edge_vals, h, h0)
    LAST_META = meta
    key = (meta["T"], tuple(meta["batches"]))
    if key not in _CACHE:
        _CACHE[key] = _build(meta)
    nc = _CACHE[key]
    res = bass_utils.run_bass_kernel_spmd(nc, in_maps, core_ids=list(range(CORES)))
    return assemble([res.results[k]["out"] for k in range(CORES)], meta)

